# revision 1
# baseline (speedup 1.0000x reference)
"""Chebyshev (L-inf) pairwise distance matrix on 8 TRN2 NeuronCores.

reference: out[i, j] = max_d |embed1[i, d] - embed2[j, d]|
  embed1: [4096, 32] f32, embed2: [4096, 32] f32, out: [4096, 4096] f32

Sharding: 8 cores = 4 i-quarters x 2 j-halves. Each core computes the
[2048 j, 1024 i] transposed block of the output.

Per-core layout: partition axis = j (16 blocks of 128), free axis = i (1024).
For each j-block and each d, the absdiff |e1[i,d] - e2[j,d]| is computed as
either an ACT activation-Abs (bias = -e2[j,d] per partition) or a DVE
tensor_scalar subtract (4x bf16 perf mode) whose sign bit is then cleared by
a single wide bitwise-and on the uint16 view. The tensor operand is e1's
column d broadcast across the 128 partitions (host-prepped, DMA'd once);
the -e2 bias table is negated on-chip from the e2 load.
The max-reduction over d is an in-place wide max tensor_tensor tree on DVE
(2x bf16 mode). Output is bf16, upcast on host. GPSIMD/PE are unusable here:
walrus rejects TensorTensor/TensorScalar on Pool for core v3, and abs_max
is not encodable at all.
"""

import sys

if "/opt/trn_rl_repo" not in sys.path:
    sys.path.insert(0, "/opt/trn_rl_repo")

from contextlib import ExitStack

import ml_dtypes
import numpy as np

import concourse.bacc as bacc
import concourse.bass as bass
import concourse.tile as tile
from concourse import mybir

BF16 = ml_dtypes.bfloat16

N = 4096          # rows of embed1 (= rows of embed2)
D = 32            # feature dim
N_CORES = 8
N_IQ = 4          # i split (embed1 rows)
N_JH = 2          # j split (embed2 rows)
I_PER = N // N_IQ       # 1024 per core
J_PER = N // N_JH       # 2048 per core
JB = J_PER // 128       # 16 j-blocks per core
U = I_PER               # free-dim elements per d-slot

# d ownership for the absdiff stage: ACT does d[0:23], DVE d[23:32].
N_ACT = 23
N_DVE = 9
assert N_ACT + N_DVE == D
# e1r_act arrives in chunks so the first ACT ops don't wait on the full 6 MB;
# tiny first chunk => ACT starts after ~0.5 MB of DMA instead of 2 MB.
ACT_CHUNKS = (2, 7, 7, 7)
assert sum(ACT_CHUNKS) == N_ACT

_nc_cache = None


def _build_nc():
    nc = bacc.Bacc(
        trn_type="TRN2",
        target_bir_lowering=False,
        debug=False,
        num_devices=N_CORES,
    )

    dt_bf16 = mybir.dt.bfloat16
    dt_u16 = mybir.dt.uint16
    dt_f32 = mybir.dt.float32

    # e1 slab transposed to d-major and broadcast across 128 partitions
    # (host side), split by absdiff owner.
    e1r_act = nc.declare_dram_parameter("e1r_act", [128, N_ACT * U], dt_bf16, isOutput=False)
    e1r_dve = nc.declare_dram_parameter("e1r_dve", [128, N_DVE * U], dt_bf16, isOutput=False)
    # e2 j-half slab [J_PER, 32] f32 (negated on-chip for the ACT bias).
    e2b = nc.declare_dram_parameter("e2b", [J_PER, D], dt_f32, isOutput=False)
    out = nc.declare_dram_parameter("out", [J_PER, I_PER], dt_bf16, isOutput=True)

    vmax = mybir.AluOpType.max
    sub = mybir.AluOpType.subtract
    band = mybir.AluOpType.bitwise_and

    with tile.TileContext(nc) as tc, ExitStack() as ctx:
        p_e1 = ctx.enter_context(tc.tile_pool(name="e1", bufs=1))
        p_e2 = ctx.enter_context(tc.tile_pool(name="e2", bufs=1))
        p_act = ctx.enter_context(tc.tile_pool(name="ract", bufs=2))
        p_dve = ctx.enter_context(tc.tile_pool(name="rdve", bufs=1))
        p_out = ctx.enter_context(tc.tile_pool(name="out", bufs=2))

        # --- one-time loads, smallest-first so both engines start early ---
        t_e2 = p_e2.tile([128, JB * D], dt_f32, tag="e2")
        t_e2n = p_e2.tile([128, JB * D], dt_f32, tag="e2n")
        e2_src = e2b[:, :].rearrange("(jb p) d -> p jb d", p=128)
        nc.sync.dma_start(t_e2[:].rearrange("p (jb d) -> p jb d", d=D), e2_src)
        # ACT bias wants -e2; negate on-chip (tiny op) instead of a 2nd DMA
        nc.vector.tensor_scalar(t_e2n[:], t_e2[:], -1.0, None,
                                op0=mybir.AluOpType.mult)

        # first ACT chunk (2 slabs) lands fast so ACT starts ~4us in; the whole
        # DVE region (9 slabs) next; remaining ACT chunks stream in behind
        t_e1a_chunks = []
        off_a = ACT_CHUNKS[0]
        t0 = p_e1.tile([128, ACT_CHUNKS[0] * U], dt_bf16, tag="e1a0")
        nc.sync.dma_start(t0[:], e1r_act[:, :off_a * U])
        t_e1a_chunks.append((0, ACT_CHUNKS[0], t0))
        t_e1d = p_e1.tile([128, N_DVE * U], dt_bf16, tag="e1d")
        nc.sync.dma_start(t_e1d[:], e1r_dve[:, :])
        for ci, csz in enumerate(ACT_CHUNKS[1:], 1):
            t = p_e1.tile([128, csz * U], dt_bf16, tag=f"e1a{ci}")
            nc.sync.dma_start(t[:], e1r_act[:, off_a * U:(off_a + csz) * U])
            t_e1a_chunks.append((off_a, csz, t))
            off_a += csz

        def emit_block(jb, i_lo, w, seq_ract=False):
            """absdiff + reduce + store for j-block jb, i-range [i_lo, i_lo+w)."""
            r_a = p_act.tile([128, N_ACT * w], dt_bf16, tag="ract")
            r_d = p_dve.tile([128, N_DVE * w], dt_bf16, tag="rdve")

            # --- absdiff stage ---
            # DVE's independent work first (keeps DVE busy while ACT runs)
            for k in range(N_DVE):
                d = N_ACT + k
                # raw diff; abs happens in the wide sign-clear below
                nc.vector.tensor_scalar(
                    r_d[:, k * w:(k + 1) * w],
                    t_e1d[:, k * U + i_lo:k * U + i_lo + w],
                    t_e2[:, jb * D + d: jb * D + d + 1],
                    None,
                    op0=sub,
                )
            # clear bf16 sign bits of the whole DVE region in one wide op
            r_d_u16 = r_d[:].bitcast(dt_u16)
            nc.vector.tensor_scalar(r_d_u16, r_d_u16, 0x7FFF, None, op0=band)
            for off, csz, t in t_e1a_chunks:
                for kk in range(csz):
                    k = off + kk
                    d = k
                    # out = Abs(in * 1.0 + (-e2col))
                    nc.scalar.activation(
                        r_a[:, k * w:(k + 1) * w],
                        t[:, kk * U + i_lo:kk * U + i_lo + w],
                        mybir.ActivationFunctionType.Abs,
                        bias=t_e2n[:, jb * D + d: jb * D + d + 1],
                        scale=1.0,
                    )

            # --- reduction: in-place wide max trees (DVE) ---
            # DVE region: 9 slots -> 4 -> 2 -> 1 (+ ragged 9th)
            nc.vector.tensor_tensor(r_d[:, :4 * w], r_d[:, :4 * w], r_d[:, 4 * w:8 * w], op=vmax)
            nc.vector.tensor_tensor(r_d[:, :2 * w], r_d[:, :2 * w], r_d[:, 2 * w:4 * w], op=vmax)
            nc.vector.tensor_tensor(r_d[:, :w], r_d[:, :w], r_d[:, w:2 * w], op=vmax)
            nc.vector.tensor_tensor(r_d[:, :w], r_d[:, :w], r_d[:, 8 * w:9 * w], op=vmax)
            if seq_ract:
                # last block: narrow sequential accumulation — each max op
                # chases the matching ACT absdiff, so after ACT's final slab
                # only ~1 op remains (shrinks the kernel-tail bubble)
                for k in range(1, N_ACT):
                    nc.vector.tensor_tensor(r_a[:, :w], r_a[:, :w],
                                            r_a[:, k * w:(k + 1) * w], op=vmax)
            else:
                # ACT region: 23 slots -> 8(+7) -> 4 -> 2 -> 1
                nc.vector.tensor_tensor(r_a[:, :8 * w], r_a[:, :8 * w], r_a[:, 8 * w:16 * w], op=vmax)
                nc.vector.tensor_tensor(r_a[:, :7 * w], r_a[:, :7 * w], r_a[:, 16 * w:23 * w], op=vmax)
                nc.vector.tensor_tensor(r_a[:, :4 * w], r_a[:, :4 * w], r_a[:, 4 * w:8 * w], op=vmax)
                nc.vector.tensor_tensor(r_a[:, :2 * w], r_a[:, :2 * w], r_a[:, 2 * w:4 * w], op=vmax)
                nc.vector.tensor_tensor(r_a[:, :w], r_a[:, :w], r_a[:, w:2 * w], op=vmax)

            # --- final merge + store ---
            t_out = p_out.tile([128, w], dt_bf16, tag="out")
            nc.vector.tensor_tensor(t_out[:], r_a[:, :w], r_d[:, :w], op=vmax)

            nc.sync.dma_start(out[jb * 128:(jb + 1) * 128, i_lo:i_lo + w], t_out[:])

        for jb in range(JB):
            emit_block(jb, 0, U)

    nc.finalize()
    return nc


def _get_nc():
    global _nc_cache
    if _nc_cache is None:
        _nc_cache = _build_nc()
    return _nc_cache


def make_in_maps(embed1: np.ndarray, embed2: np.ndarray):
    """Host-side sharding/prep. Returns in_maps for cores 0..7.

    Core c: iq = c % N_IQ, jh = c // N_IQ.
    """
    embed1 = np.asarray(embed1, dtype=np.float32)
    embed2 = np.asarray(embed2, dtype=np.float32)
    in_maps = []
    for c in range(N_CORES):
        iq, jh = c % N_IQ, c // N_IQ
        e1_slab = embed1[iq * I_PER:(iq + 1) * I_PER, :]      # [1024, 32]
        # d-major flatten, bf16, broadcast to 128 partitions
        flat = np.ascontiguousarray(e1_slab.T).reshape(-1).astype(BF16)  # [32*1024]
        rep = np.ascontiguousarray(np.broadcast_to(flat[None, :], (128, D * I_PER)))
        e2_slab = np.ascontiguousarray(embed2[jh * J_PER:(jh + 1) * J_PER, :])  # [2048, 32]
        in_maps.append({
            "e1r_act": np.ascontiguousarray(rep[:, :N_ACT * U]),
            "e1r_dve": np.ascontiguousarray(rep[:, N_ACT * U:]),
            "e2b": e2_slab,
        })
    return in_maps


def assemble(results) -> np.ndarray:
    """results: list of per-core dicts with 'out' [J_PER, I_PER] bf16."""
    full = np.empty((N, N), dtype=np.float32)
    for c in range(N_CORES):
        iq, jh = c % N_IQ, c // N_IQ
        blk = np.asarray(results[c]["out"]).astype(np.float32)  # [2048, 1024]
        full[iq * I_PER:(iq + 1) * I_PER, jh * J_PER:(jh + 1) * J_PER] = blk.T
    return full


def kernel(embed1: np.ndarray, embed2: np.ndarray) -> np.ndarray:
    from concourse.bass_utils import run_bass_kernel_spmd

    nc = _get_nc()
    in_maps = make_in_maps(np.asarray(embed1), np.asarray(embed2))
    res = run_bass_kernel_spmd(nc, in_maps, core_ids=list(range(N_CORES)))
    return assemble(res.results)


if __name__ == "__main__":
    e1 = np.random.randn(N, D).astype(np.float32)
    e2 = np.random.randn(N, D).astype(np.float32)
    out = kernel(embed1=e1, embed2=e2)
    ref = np.max(np.abs(e1[:, None, :] - e2[None, :, :]), axis=2)
    err = np.abs(out - ref).max() / np.abs(ref).max()
    print("rel err:", err)

